# revision 20
# baseline (speedup 1.0000x reference)
"""Trainium2 Bass kernel: causal multi-head attention block (B=2,S=2048,H=2048,NH=16,HD=128).

Sharding: 8 cores = DP over batch (2) x TP over head-groups (4 groups of 4 heads).
Each core computes q/k/v projections for its 4 heads, RoPE, causal softmax
attention, and a partial output projection; the host sums the 4 partials per
batch and adds bo.

v3 design notes (evidence: NTFF profiles of v1/v2):
  - all inputs are pre-arranged on the host into the exact SBUF layout
    ([partition, free] contiguous), so every DMA is 128 descriptors of
    4-16KB. v2's 1KB-row descriptors capped the queue at ~52GB/s and starved
    phase 1 (77us of PE stalls).
  - single x pass: V projection reuses the resident x tiles.
  - startup interleaves 2-f-tile chunks of wq/wk/x so the first matmul fires
    after ~1.2MB.
  - RoPE runs bf16 end-to-end (ACT drains rotation PSUM -> bf16 SBUF; DVE
    elementwise in 16-bit mode).
  - attention is software-pipelined with a 2-tile lookahead (st/exp run 2
    tiles ahead of pv/dn), hiding the exp latency.
  - 1/denominator is broadcast across partitions by GPSIMD partition_broadcast
    (saves the K=1 PE matmul + two DVE ops of v1/v2).
  - the output projection of q-block j is deferred and interleaved into the
    attention stream of block j+1, so the PE never waits for the DVE
    normalize chain; output tiles assemble in [128, 2048] SBUF blocks
    (drains alternate ACT/DVE) and ship as one 4KB-per-partition bf16 DMA.
"""

import math
import os
import sys

import numpy as np

for _p in ("/opt/trn_rl_repo",):
    if _p not in sys.path and os.path.isdir(_p):
        sys.path.insert(0, _p)

import ml_dtypes

import concourse.bass as bass
import concourse.mybir as mybir
import concourse.tile as tile
from concourse import bacc

B, S, H, NH, HD = 2, 2048, 2048, 16, 128
NCORES = 8
HG = 4            # head-groups (TP degree)
HPG = NH // HG    # heads per group = 4
DLOC = HPG * HD   # local d width = 512
FT = H // 128     # 16 f-tiles
SJ = S // 512     # 4 s/q tiles of 512
KT128 = S // 128  # 16 k-tiles of 128
NEG = -1e30

F32 = mybir.dt.float32
F16 = mybir.dt.float16
BF16 = mybir.dt.bfloat16
NPBF16 = ml_dtypes.bfloat16


def build_program(mode: str) -> bass.Bass:
    """mode in {'causal', 'full', 'bias'}"""
    import concourse.tile_sem_assignment as tsa

    tsa.NUM_HWDGE_SEMS = 1
    tsa.NUM_SWDGE_GLOBAL_SEMS = 1
    nc = bacc.Bacc()
    # host pre-arranged to SBUF layout: [partition, ...free] contiguous
    xp = nc.dram_tensor("xp", [128, SJ, FT, 512], BF16, kind="ExternalInput")
    wqp = nc.dram_tensor("wqp", [128, FT, DLOC], BF16, kind="ExternalInput")
    wkp = nc.dram_tensor("wkp", [128, FT, DLOC], BF16, kind="ExternalInput")
    wvp = nc.dram_tensor("wvp", [128, FT, DLOC], BF16, kind="ExternalInput")
    wop = nc.dram_tensor("wop", [128, HPG, H], BF16, kind="ExternalInput")
    bqT = nc.dram_tensor("bqT", [128, HPG], F32, kind="ExternalInput")
    bkT = nc.dram_tensor("bkT", [128, HPG], F32, kind="ExternalInput")
    bv = nc.dram_tensor("bv", [128, DLOC], F32, kind="ExternalInput")
    cosT = nc.dram_tensor("cosT", [HD, S], BF16, kind="ExternalInput")
    sinT = nc.dram_tensor("sinT", [HD, S], BF16, kind="ExternalInput")
    rmat = nc.dram_tensor("rmat", [HD, HD], BF16, kind="ExternalInput")
    if mode == "causal":
        dbp = nc.dram_tensor("dbp", [128, 4, 512], F32, kind="ExternalInput")
    elif mode == "bias":
        fbias = nc.dram_tensor("fbias", [S, S], F32, kind="ExternalInput")
    # output tiled: y2[qj, ss, p, o] = y[qj*512 + ss*128 + p, o]
    y2 = nc.dram_tensor("y2", [SJ, 4, 128, H], BF16, kind="ExternalOutput")

    with tile.TileContext(nc) as tc:
        with (
            tc.tile_pool(name="qt", bufs=HPG * SJ) as qt_pool,
            tc.tile_pool(name="kt", bufs=HPG * SJ) as kt_pool,
            tc.tile_pool(name="vt", bufs=KT128) as vt_pool,
            tc.tile_pool(name="consts", bufs=1) as consts,
        ):
            QT = {}  # (h, sj) -> [128, 512] bf16 tile, RoPE'd q^T (pre-scaled)
            KT = {}  # (h, sj) -> [128, 512] bf16 tile, RoPE'd k^T
            VT = {}  # ssub -> [128(s), 512(d)] bf16 tile, v + bias

            ones_sb = consts.tile([128, 1], BF16, tag="ones")
            nc.gpsimd.memset(ones_sb[:], 1.0)
            wz_sb = consts.tile([128, 64], BF16, tag="wz")
            nc.gpsimd.memset(wz_sb[:], 0.0)
            bv_sb = consts.tile([128, DLOC], F32, tag="bv")
            wo_sb = consts.tile([128, HPG, H], BF16, tag="wo")
            db_sb = None
            if mode == "causal":
                db_sb = consts.tile([128, 4, 512], F32, tag="db")

            # ============ Phase 1: Q/K/V projections + RoPE (one x pass) ====
            with (
                tc.tile_pool(name="ps12", bufs=8, space="PSUM") as psum,
                tc.tile_pool(name="wqk", bufs=1) as wqk_pool,
                tc.tile_pool(name="csn", bufs=1) as csn_pool,
                tc.tile_pool(name="xin", bufs=SJ) as xin_pool,
                tc.tile_pool(name="rtmp", bufs=3) as rtmp_pool,
                tc.tile_pool(name="rsb", bufs=3) as rsb_pool,
            ):
                wq_sb = wqk_pool.tile([128, FT, DLOC], BF16, tag="wq")
                wk_sb = wqk_pool.tile([128, FT, DLOC], BF16, tag="wk")
                wv_sb = wqk_pool.tile([128, FT, DLOC], BF16, tag="wv")
                cos_sb = csn_pool.tile([HD, S], BF16, tag="cos")
                sin_sb = csn_pool.tile([HD, S], BF16, tag="sin")
                rmat_sb = consts.tile([HD, HD], BF16, tag="rmat")
                bq_sb = consts.tile([128, HPG], F32, tag="bq")
                bk_sb = consts.tile([128, HPG], F32, tag="bk")

                # PE warmup: ~120 dependency-free matmuls on memset tiles run
                # while the first DMAs stream in, releasing the HAM clock gate
                # (K=8/8 after ~4us of activity) so the real q-pass starts at
                # full clock instead of ramping through the cold 1.2GHz state.
                warm_ps = psum.tile([128, 512], F32, tag="ps", name="warm")
                for _ in range(170):
                    nc.tensor.matmul(
                        warm_ps[:64, :64], wz_sb[:, 0:64], wz_sb[:, 0:64],
                        start=True, stop=True,
                    )
                warm_rd = rsb_pool.tile([64, 64], BF16, tag="wr", name="wr")
                nc.scalar.copy(warm_rd[:], warm_ps[:64, :64])

                xts = []
                xt0 = xin_pool.tile([128, FT, 512], BF16, tag="xt", name="xt")
                xts.append(xt0)
                # startup: 8-f-tile halves (8KB/partition descriptors run at
                # ~250GB/s; finer chunks fall off the descriptor-rate cliff).
                # q-pass weights + x first; wk lands during the q matmuls.
                h1, h2 = slice(0, 8), slice(8, 16)
                nc.sync.dma_start(wq_sb[:, h1, :], wqp[:, h1, :])
                nc.sync.dma_start(xt0[:, h1, :], xp[:, 0, h1, :])
                nc.sync.dma_start(wq_sb[:, h2, :], wqp[:, h2, :])
                nc.sync.dma_start(xt0[:, h2, :], xp[:, 0, h2, :])
                nc.sync.dma_start(rmat_sb[:], rmat[:])
                nc.sync.dma_start(bq_sb[:], bqT[:])
                nc.sync.dma_start(bk_sb[:], bkT[:])
                nc.sync.dma_start(wk_sb[:, h1, :], wkp[:, h1, :])
                nc.sync.dma_start(wk_sb[:, h2, :], wkp[:, h2, :])
                nc.sync.dma_start(cos_sb[:], cosT[:])
                nc.sync.dma_start(sin_sb[:], sinT[:])
                # remaining tiles in need order (16KB/partition each)
                xt1 = xin_pool.tile([128, FT, 512], BF16, tag="xt", name="xt")
                xts.append(xt1)
                nc.sync.dma_start(xt1[:], xp[:, 1])
                nc.sync.dma_start(wv_sb[:], wvp[:])
                nc.sync.dma_start(bv_sb[:], bv[:])
                for sj in (2, 3):
                    xt = xin_pool.tile([128, FT, 512], BF16, tag="xt", name="xt")
                    xts.append(xt)
                    nc.sync.dma_start(xt[:], xp[:, sj])
                nc.sync.dma_start(wo_sb[:], wop[:])
                if mode == "causal":
                    nc.sync.dma_start(db_sb[:], dbp[:])

                for sj in range(SJ):
                    xt = xts[sj]
                    # split q-pass then k-pass: on sj=0 the k weights stream
                    # in while the q matmuls run.
                    qp = [psum.tile([128, 512], F32, tag="ps", name="ps") for _ in range(HPG)]
                    kp = [psum.tile([128, 512], F32, tag="ps", name="ps") for _ in range(HPG)]
                    for ft in range(FT):
                        for h in range(HPG):
                            nc.tensor.matmul(
                                qp[h][:],
                                wq_sb[:, ft, h * 128 : (h + 1) * 128],
                                xt[:, ft, :],
                                start=(ft == 0),
                                stop=(ft == FT - 1),
                            )
                    for ft in range(FT):
                        for h in range(HPG):
                            nc.tensor.matmul(
                                kp[h][:],
                                wk_sb[:, ft, h * 128 : (h + 1) * 128],
                                xt[:, ft, :],
                                start=(ft == 0),
                                stop=(ft == FT - 1),
                            )
                    css = cos_sb[:, sj * 512 : (sj + 1) * 512]
                    sss = sin_sb[:, sj * 512 : (sj + 1) * 512]
                    # drain all 8 PSUM banks on ACT first (bias fold + bf16)
                    work = []
                    for h in range(HPG):
                        for ps, bias_sb, pool, store in (
                            (qp[h], bq_sb, qt_pool, QT),
                            (kp[h], bk_sb, kt_pool, KT),
                        ):
                            t = pool.tile([128, 512], BF16, tag="t", name="qkt")
                            nc.scalar.activation(
                                t[:],
                                ps[:],
                                mybir.ActivationFunctionType.Identity,
                                bias=bias_sb[:, h : h + 1],
                            )
                            work.append((h, t, store))
                    # rotate-half via permutation matmul; ACT drains to bf16
                    # so the three DVE ops run in 16-bit mode.
                    for h, t, store in work:
                        rp = psum.tile([128, 512], F32, tag="ps", name="ps")
                        nc.tensor.matmul(rp[:], rmat_sb[:], t[:], start=True, stop=True)
                        r_sb = rsb_pool.tile([128, 512], BF16, tag="r", name="r")
                        nc.scalar.copy(r_sb[:], rp[:])
                        tmp = rtmp_pool.tile([128, 512], BF16, tag="tmp", name="tmp")
                        nc.vector.tensor_mul(tmp[:], r_sb[:], sss)
                        nc.vector.tensor_mul(t[:], t[:], css)
                        nc.vector.tensor_add(t[:], t[:], tmp[:])
                        store[(h, sj)] = t

                # V projection from the resident x tiles (8 banks per wave)
                for grp in range(2):
                    vps = [psum.tile([128, 512], F32, tag="ps", name="ps") for _ in range(8)]
                    for ft in range(FT):
                        for i in range(8):
                            ss = grp * 8 + i
                            nc.tensor.matmul(
                                vps[i][:],
                                xts[ss // 4][:, ft, (ss % 4) * 128 : (ss % 4 + 1) * 128],
                                wv_sb[:, ft, :],
                                start=(ft == 0),
                                stop=(ft == FT - 1),
                            )
                    for i in range(8):
                        ss = grp * 8 + i
                        v = vt_pool.tile([128, DLOC], BF16, tag="v", name="v")
                        nc.vector.tensor_add(v[:], vps[i][:], bv_sb[:])
                        VT[ss] = v

            # ============ Phase 3: attention + output projection ============
            with (
                tc.tile_pool(name="pst", bufs=2, space="PSUM") as psum_st,
                tc.tile_pool(name="ppv", bufs=2, space="PSUM") as psum_pv,
                tc.tile_pool(name="pdn", bufs=2, space="PSUM") as psum_dn,
                tc.tile_pool(name="pyp", bufs=2, space="PSUM") as psum_yp,
                tc.tile_pool(name="ex", bufs=8) as exp_pool,
                tc.tile_pool(name="ot", bufs=2 * HPG) as ot_pool,
                tc.tile_pool(name="rc", bufs=4) as rc_pool,
                tc.tile_pool(name="ysb", bufs=8) as y_pool,
                tc.tile_pool(name="fb", bufs=3) as fb_pool,
            ):
                pending_oproj = [None]

                def make_oproj(qj, OT):
                    def emit():
                        for ss in range(4):
                            ysb = y_pool.tile([128, H], BF16, tag="y", name="y")
                            for oj in range(4):
                                yp = psum_yp.tile([128, 512], F32, tag="yp", name="yp")
                                for dt in range(HPG):
                                    nc.tensor.matmul(
                                        yp[:],
                                        OT[dt][:, ss * 128 : (ss + 1) * 128],
                                        wo_sb[:, dt, oj * 512 : (oj + 1) * 512],
                                        start=(dt == 0),
                                        stop=(dt == HPG - 1),
                                    )
                                dst = ysb[:, oj * 512 : (oj + 1) * 512]
                                if oj % 2 == 0:
                                    nc.scalar.copy(dst, yp[:])
                                else:
                                    nc.vector.tensor_copy(dst, yp[:])
                            nc.sync.dma_start(y2[qj, ss], ysb[:])
                    return emit

                for qj in range(SJ):
                    kmax = 4 * qj + 4 if mode == "causal" else KT128
                    OT = {}
                    PV = {}
                    DN = {}
                    RC = {}
                    ES = {}

                    def _off(kj):
                        a = kj - 4 * qj
                        return 128 * a if (mode == "causal" and a > 0) else 0

                    def _issue_st(h, kj):
                        off = _off(kj)
                        st = psum_st.tile([128, 512], F32, tag="st", name="st")
                        nc.tensor.matmul(
                            st[:, off:],
                            KT[(h, kj // 4)][:, (kj % 4) * 128 : (kj % 4 + 1) * 128],
                            QT[(h, qj)][:, off:],
                            start=True,
                            stop=True,
                        )
                        a = kj - 4 * qj
                        if mode == "causal" and a >= 0:
                            nc.vector.tensor_add(
                                st[:, off : off + 128],
                                st[:, off : off + 128],
                                db_sb[:, a, off : off + 128],
                            )
                        elif mode == "bias":
                            fbt = fb_pool.tile([128, 512], F32, tag="fb", name="fb")
                            nc.sync.dma_start(
                                fbt[:],
                                fbias[
                                    kj * 128 : (kj + 1) * 128,
                                    qj * 512 : (qj + 1) * 512,
                                ],
                            )
                            nc.vector.tensor_add(st[:], st[:], fbt[:])
                        e = exp_pool.tile([128, 512], BF16, tag="e", name="e")
                        nc.scalar.activation(
                            e[:, off:], st[:, off:],
                            mybir.ActivationFunctionType.Exp,
                        )
                        ES[(h, kj)] = e

                    def _normalize(i):
                        # 1/denom broadcast across partitions on GPSIMD (idle
                        # engine; latency hidden by the deferred schedule)
                        rcb = rc_pool.tile([128, 512], F32, tag="rcb", name="rcb")
                        nc.gpsimd.partition_broadcast(rcb[:], RC[i][:], channels=128)
                        ot = ot_pool.tile([128, 512], BF16, tag="ot", name="ot")
                        nc.vector.tensor_mul(ot[:], PV[i][:], rcb[:])
                        OT[i] = ot

                    def _issue_pvdn(h, kj):
                        off = _off(kj)
                        e = ES.pop((h, kj))
                        if kj == 0:
                            PV[h] = psum_pv.tile([128, 512], F32, tag="pv", name="pv")
                            DN[h] = psum_dn.tile([1, 512], F32, tag="dn", name="dn")
                        nc.tensor.matmul(
                            PV[h][:, off:],
                            VT[kj][:, h * 128 : (h + 1) * 128],
                            e[:, off:],
                            start=(kj == 0),
                            stop=(kj == kmax - 1),
                        )
                        nc.tensor.matmul(
                            DN[h][:, off:],
                            ones_sb[:],
                            e[:, off:],
                            start=(kj == 0),
                            stop=(kj == kmax - 1),
                        )
                        if kj == kmax - 1:
                            rcf = rc_pool.tile([1, 512], F32, tag="rcf", name="rcf")
                            nc.vector.reciprocal_approx_fast(rcf[:], DN[h][:])
                            RC[h] = rcf
                            if h > 0:
                                _normalize(h - 1)
                            if h == HPG - 1:
                                _normalize(h)

                    seq = [(h, kj) for h in range(HPG) for kj in range(kmax)]
                    LOOK = 2
                    for i, (h, kj) in enumerate(seq):
                        _issue_st(h, kj)
                        if i >= LOOK:
                            _issue_pvdn(*seq[i - LOOK])
                        # previous q-block's o-projection slots in here: the
                        # PE chews it while this block's exps stream on ACT.
                        if i == 6 and pending_oproj[0] is not None:
                            pending_oproj[0]()
                            pending_oproj[0] = None
                    for i in range(len(seq) - LOOK, len(seq)):
                        _issue_pvdn(*seq[i])
                    if pending_oproj[0] is not None:  # safety (never for SJ>1)
                        pending_oproj[0]()
                    pending_oproj[0] = make_oproj(qj, OT)
                pending_oproj[0]()
    nc.compile()
    return nc


_PROGRAM_CACHE = {}


def _get_program(mode):
    if mode not in _PROGRAM_CACHE:
        _PROGRAM_CACHE[mode] = build_program(mode)
    return _PROGRAM_CACHE[mode]


def _detect_mode(attn_mask):
    m = np.asarray(attn_mask).reshape(S, S)
    if (m == np.tril(np.ones((S, S), m.dtype))).all():
        return "causal"
    if (m != 0).all():
        return "full"
    return "bias"


def _rot_matrix():
    # rot(q)[d'] = -q[d'+1] (d' even), +q[d'-1] (d' odd);  rotT = R^T @ qT with
    # lhsT[d, d'] convention of nc.tensor.matmul.
    r = np.zeros((HD, HD), np.float32)
    for dp in range(HD):
        if dp % 2 == 0:
            r[dp + 1, dp] = -1.0
        else:
            r[dp - 1, dp] = 1.0
    return r


def _diag_bias():
    # [128(p), 4(a), 512(t)]: 0 where 128a+p <= t else -1e30
    a = np.arange(4)[None, :, None]
    p = np.arange(128)[:, None, None]
    t = np.arange(512)[None, None, :]
    return np.where(128 * a + p <= t, 0.0, NEG).astype(np.float32)


def _bf16(a):
    return np.ascontiguousarray(a).astype(NPBF16)


def _prep_w(wT):
    # [H, DLOC] -> [128, FT, DLOC] with [p, ft, d] = wT[ft*128+p, d]
    return np.ascontiguousarray(wT.reshape(FT, 128, DLOC).transpose(1, 0, 2))


def kernel(**inputs) -> np.ndarray:
    from concourse.bass_utils import run_bass_kernel_spmd

    x = np.asarray(inputs["x"], np.float32)
    fcos = np.asarray(inputs["fcos"], np.float32)
    fsin = np.asarray(inputs["fsin"], np.float32)
    Wq, bq = np.asarray(inputs["Wq"], np.float32), np.asarray(inputs["bq"], np.float32)
    Wk, bk = np.asarray(inputs["Wk"], np.float32), np.asarray(inputs["bk"], np.float32)
    Wv, bv = np.asarray(inputs["Wv"], np.float32), np.asarray(inputs["bv"], np.float32)
    Wo, bo = np.asarray(inputs["Wo"], np.float32), np.asarray(inputs["bo"], np.float32)
    attn_mask = inputs["attn_mask"]

    mode = _detect_mode(attn_mask)
    nc = _get_program(mode)

    sc = 1.0 / math.sqrt(HD)
    shared = {
        "cosT": _bf16(fcos.T),
        "sinT": _bf16(fsin.T),
        "rmat": _rot_matrix().astype(NPBF16),
    }
    if mode == "causal":
        shared["dbp"] = _diag_bias()
    elif mode == "bias":
        m = np.asarray(attn_mask).reshape(S, S)
        shared["fbias"] = np.ascontiguousarray(
            np.where(m.T == 0, NEG, 0.0).astype(np.float32)
        )

    in_maps = []
    for c in range(NCORES):
        b, hg = divmod(c, HG)
        rows = slice(DLOC * hg, DLOC * (hg + 1))
        xT = x[b].T  # [H, S]
        # [128, SJ, FT, 512]: [p, sj, ft, s] = xT[ft*128+p, sj*512+s]
        xprep = xT.reshape(FT, 128, SJ, 512).transpose(1, 2, 0, 3)
        woT = Wo[:, rows].T  # [DLOC, H]
        wo_prep = woT.reshape(HPG, 128, H).transpose(1, 0, 2)
        in_maps.append(
            {
                "xp": _bf16(xprep),
                "wqp": _bf16(_prep_w((Wq[rows] * sc).T)),
                "wkp": _bf16(_prep_w(Wk[rows].T)),
                "wvp": _bf16(_prep_w(Wv[rows].T)),
                "wop": _bf16(wo_prep),
                "bqT": np.ascontiguousarray((bq[rows] * sc).reshape(HPG, 128).T),
                "bkT": np.ascontiguousarray(bk[rows].reshape(HPG, 128).T),
                "bv": np.ascontiguousarray(
                    np.broadcast_to(bv[rows].reshape(1, DLOC), (128, DLOC))
                ).astype(np.float32),
                **shared,
            }
        )

    trace = bool(int(os.environ.get("KERNEL_TRACE", "0")))
    res = run_bass_kernel_spmd(nc, in_maps, list(range(NCORES)), trace=trace)
    if trace and res.exec_time_ns is not None:
        print(f"HW exec time: {res.exec_time_ns} ns")
        globals()["LAST_EXEC_NS"] = res.exec_time_ns
        globals()["LAST_RESULTS"] = res

    out = np.zeros((B, S, H), np.float32)
    for c in range(NCORES):
        yt = np.asarray(res.results[c]["y2"]).astype(np.float32)  # [SJ,4,128,H]
        out[c // HG] += yt.reshape(S, H)
    out += bo
    return out
